# revision 15
# baseline (speedup 1.0000x reference)
"""BatchBlur: depthwise 15x15 conv with per-sample kernels, reflection pad 7.

x: (32, 3, 512, 512) f32, kernel: (32, 15, 15) f32 -> out (32, 3, 512, 512) f32.

Strategy: pure data parallel over batch, 4 samples (12 channel-images) per
core on 8 cores.  Host reflection-pads to (., 526, 528) fp16.

Device formulation (v2 -- triple-band, 4-way column tiling):
The 128x128 PE array is column-tiled into four 32-wide groups, one image
per group.  Each rhs tile holds a strip of 42 input rows THREE times in
partitions 0:42 / 42:84 / 84:126, at column shifts +0 / +1 / +2.  The
stationary matrix A[k=42*b+r, m] = kern[r-m, 3j+b] (band 0<=r-m<15) then
makes ONE accumulating matmul cover THREE horizontal taps for 28 output
rows, so five passes (j=0..4, rhs offset 3j) cover all 15 taps:
  out[m, n] += sum_k A[k, j, m] * rhs[k, n + 3j]
Per strip-unit: 5 passes x 4 images of N=512 -> 20 matmuls but only
5 x 512 PE-streaming cycles (vs 8 x 512 per 2 images in the dual-band
scheme): ~1.85x fewer tensor cycles, and the kernel is tensor-bound.

Band 0 comes from HBM (one 4D DMA per double-strip on the Sync queue);
band 1 (+1 col, odd byte offset) is an SBUF->SBUF DMA on the GpSimd
queue; band 2 (+2 cols, 4B aligned so the DVE 4x copy mode applies) is a
vector tensor_copy.  PSUM eviction (f32 -> fp16) runs on the Scalar/ACT
engine, whose HWDGE ring also carries the stores, so the DVE, ACT, Sync
and GpSimd queues each stay under the ~1.07us/strip tensor budget.

Outputs are stored raw as [3 groups, 19 strips, 128 partitions, 512]
(partition 32*i+m = image 4g+i, row r0+m; m=28..31 junk) and the host
gathers them back -- this keeps every store a single 3D-AP DMA.
Strips: r0 = 28*s for s=0..17 plus a final overlap strip at r0=484
(rows 484..525 of the padded image) whose rows 20..27 are the kept tail.
"""
import os
import sys

for _p in ("/opt/trn_rl_repo", "/root/.axon_site/_ro/trn_rl_repo"):
    if _p not in sys.path and os.path.isdir(_p):
        sys.path.insert(0, _p)

import numpy as np

import concourse.bass as bass
import concourse.mybir as mybir
import concourse.tile as tile
from concourse import bacc
from concourse.bass_utils import run_bass_kernel_spmd

L = 15           # blur kernel size
P = L // 2       # reflection pad
B, C, H, W = 32, 3, 512, 512
N_CORES = 8
BS = B // N_CORES            # samples per core
NIMG = BS * C                # channel images per core
HP, WP = H + 2 * P, W + 2 * P  # 526
WPH = WP + 2                 # host row pitch: 528 (two defined pad columns)
M_STRIP = 28                 # output rows per strip
KB = M_STRIP + L - 1         # 42-row band
NB = 3                       # bands (column shifts 0,1,2)
NP = 5                       # passes: dx = 3j+b, j=0..4
NG = NIMG // 4               # 3 groups of 4 column-tiled images
NS = 19                      # strips: 18 at r0=28s + final overlap at 484
R0_LAST = HP - KB            # 484
N_WARMUP = 70                # HAM warm-up matmuls

F16 = mybir.dt.float16
F32 = mybir.dt.float32

_program_cache = None


def _build_program():
    nc = bacc.Bacc("TRN2", target_bir_lowering=False, debug=False)
    # host-staged band-0 rows, already in per-double-unit tile layout:
    # [group, du, 42 rows, {2 strips x 4 images} x 528 cols] -- each load
    # is then a single fully-coalesced 2D DMA (du 9 uses blocks 0..3)
    xq_d = nc.dram_tensor("xq", [NG, 10, KB, 8 * WPH], F16,
                          kind="ExternalInput").ap()
    a_d = nc.dram_tensor("a", [BS, NB * KB, NP, M_STRIP], F16,
                         kind="ExternalInput").ap()
    out_d = nc.dram_tensor("out", [NG, NS, 128, W], F16,
                           kind="ExternalOutput").ap()

    def load_du(t, g, du):
        nc.sync.dma_start(out=t[64:64 + KB, :], in_=xq_d[g, du])

    def load_su(t, g):
        nc.sync.dma_start(out=t[64:64 + KB, :],
                          in_=xq_d[g, 9][:, 0:4 * WPH])

    def make_bands(t, nblk):
        # Fabric writes are the binding resource, so band 2 rides the DVE
        # (engine ops need 32-aligned partition bases: src 64, dst 0; the
        # +2-element shift keeps 4B alignment, and a uint32 bitcast halves
        # the element count for the 2x single-src mode).  Band 1 (+1 col,
        # odd offset) stays on the GpSimd DMA ring, split into the two
        # leftover partition windows 42..63 / 106..125.
        v0 = t[64:64 + KB, :].rearrange("p (b c) -> p b c", c=WPH)
        v2 = t[0:KB, :].rearrange("p (b c) -> p b c", c=WPH)
        nc.vector.tensor_copy(
            out=v2[:, :, 0:524].bitcast(mybir.dt.uint32),
            in_=v0[:, :, 2:526].bitcast(mybir.dt.uint32))
        v0a = t[64:86, :].rearrange("p (b c) -> p b c", c=WPH)
        v0b = t[86:106, :].rearrange("p (b c) -> p b c", c=WPH)
        v1a = t[42:64, :].rearrange("p (b c) -> p b c", c=WPH)
        v1b = t[106:126, :].rearrange("p (b c) -> p b c", c=WPH)
        nc.gpsimd.dma_start(out=v1a[:, :, 0:WP], in_=v0a[:, :, 1:WP + 1])
        nc.gpsimd.dma_start(out=v1b[:, :, 0:WP], in_=v0b[:, :, 1:WP + 1])

    def strip_mms(acc, x_t, a_t, samples, blk0, j_off):
        # 5 passes x 4 column-tiled images; K=126, M=28, N=512 each
        for j in range(NP):
            for i in range(4):
                col = (blk0 + i) * WPH + 3 * j + j_off
                nc.tensor.matmul(
                    acc[32 * i:32 * i + M_STRIP],
                    a_t[samples[i]][:, j, :],
                    x_t[0:NB * KB, col:col + W],
                    start=(j == 0),
                    stop=(j == NP - 1),
                    tile_position=(0, 32 * i),
                )

    with tile.TileContext(nc) as tc:
        with (
            tc.tile_pool(name="aconst", bufs=1) as apool,
            tc.tile_pool(name="warm", bufs=1) as wpool,
            tc.tile_pool(name="xin", bufs=10) as xpool,
            tc.tile_pool(name="oout", bufs=6) as opool,
            tc.tile_pool(name="psum", bufs=7, space="PSUM") as psum,
            tc.tile_pool(name="psumw", bufs=1, space="PSUM") as psumw,
        ):
            # HAM warm-up burst while the first loads are in flight
            wsrc = wpool.tile([128, 64], mybir.dt.bfloat16)
            nc.vector.memset(wsrc[:], 0.0)
            wacc = psumw.tile([64, 64], F32)
            for _ in range(N_WARMUP):
                nc.tensor.matmul(wacc[:], wsrc[:, :64], wsrc[:], start=True,
                                 stop=True)

            # first double-unit's rows: issued before the A loads so the
            # Sync queue delivers the first matmuls' dependency earliest
            x_first = xpool.tile([128, 2 * 4 * WPH], F16, tag="xdu",
                                 name="xf")
            load_du(x_first, 0, 0)

            # per-sample stationary matrices ride the otherwise-idle
            # Scalar ring (stores only start ~3us in)
            a_t = [
                apool.tile([NB * KB, NP, M_STRIP], F16, tag=f"a{s}",
                           name=f"a{s}")
                for s in range(BS)
            ]
            for s in range(BS):
                nc.scalar.dma_start(out=a_t[s][:], in_=a_d[s])

            for g in range(NG):
                samples = [(4 * g + i) // C for i in range(4)]
                for du in range(10):
                    if du < 9:
                        if g == 0 and du == 0:
                            x_t = x_first
                        else:
                            x_t = xpool.tile([128, 2 * 4 * WPH], F16,
                                             tag="xdu", name="xdu")
                            load_du(x_t, g, du)
                        make_bands(x_t, 8)
                        o_t = opool.tile([128, 2 * W], F16, tag="odu",
                                         name="odu")
                        for q in range(2):
                            acc = psum.tile([128, W], F32)
                            strip_mms(acc, x_t, a_t, samples, 4 * q, 0)
                            # quarter-col-tiled matmuls don't register as
                            # PE activity in the HAM clock gate; one
                            # full-row h0 heartbeat per strip keeps K=8/8
                            nc.tensor.matmul(wacc[:], wsrc[:, :64],
                                             wsrc[:], start=True, stop=True)
                            # PSUM f32 -> SBUF fp16 on the ACT engine
                            nc.scalar.copy(out=o_t[:, q * W:(q + 1) * W],
                                           in_=acc[:])
                        s0 = 2 * du
                        nc.scalar.dma_start(
                            out=out_d[g, s0:s0 + 2].rearrange(
                                "q p c -> p q c"),
                            in_=o_t[:, :].rearrange("p (q c) -> p q c",
                                                    c=W))
                    else:
                        x_t = xpool.tile([128, 4 * WPH], F16, tag="xsu",
                                         name="xsu")
                        load_su(x_t, g)
                        make_bands(x_t, 4)
                        o_t = opool.tile([128, W], F16, tag="osu",
                                         name="osu")
                        acc = psum.tile([128, W], F32)
                        strip_mms(acc, x_t, a_t, samples, 0, 0)
                        nc.tensor.matmul(wacc[:], wsrc[:, :64], wsrc[:],
                                         start=True, stop=True)
                        nc.scalar.copy(out=o_t[:], in_=acc[:])
                        nc.scalar.dma_start(out=out_d[g, 18], in_=o_t[:])
    nc.compile()
    return nc


def prepare_in_maps(x: np.ndarray, kern: np.ndarray) -> list:
    # host-side reflection pad, fp16, rows padded to WPH with zero columns
    xpc = np.pad(x, ((0, 0), (0, 0), (P, P), (P, P)), mode="reflect")
    xp = np.zeros((B * C, HP, WPH), dtype=np.float16)
    xp[:, :, :WP] = xpc.reshape(B * C, HP, WP).astype(np.float16)

    # band-0 rows restaged per double-unit: [core, g, du, 42, 8*WPH] where
    # block q*4+i = strip 56*du+28*q of image 4g+i (du 9: the r0=484
    # overlap strip in blocks 0..3)
    xv = xp.reshape(N_CORES, NG, 4, HP, WPH)
    xq = np.empty((N_CORES, NG, 10, KB, 8, WPH), dtype=np.float16)
    for du in range(9):
        for q in range(2):
            r0 = 56 * du + 28 * q
            xq[:, :, du, :, 4 * q:4 * q + 4, :] = xv[
                :, :, :, r0:r0 + KB, :].transpose(0, 1, 3, 2, 4)
    xq[:, :, 9, :, 0:4, :] = xv[:, :, :, R0_LAST:R0_LAST + KB, :].transpose(
        0, 1, 3, 2, 4)
    xq[:, :, 9, :, 4:8, :] = 0.0

    # triple-band stationary matrices; partition layout (r = band row):
    #   k =  0..41  band2 (dx=3j+2), k = 64+r   band0 (dx=3j),
    #   k = 42+r (r<22) / 84+r (r>=22)  band1 (dx=3j+1)
    kern16 = kern.astype(np.float16)
    a_all = np.zeros((B, NB * KB, NP, M_STRIP), dtype=np.float16)
    m_idx = np.arange(M_STRIP)
    for dy in range(L):
        r = m_idx + dy
        a_all[:, r, :, m_idx] = kern16[:, dy, 2::NB]
        a_all[:, 64 + r, :, m_idx] = kern16[:, dy, 0::NB]
        k1 = np.where(r < 22, 42 + r, 84 + r)
        a_all[:, k1, :, m_idx] = kern16[:, dy, 1::NB]

    return [
        {
            "xq": xq[c].reshape(NG, 10, KB, 8 * WPH),
            "a": a_all[c * BS:(c + 1) * BS],
        }
        for c in range(N_CORES)
    ]


def kernel(x: np.ndarray, kernel: np.ndarray) -> np.ndarray:
    global _program_cache
    x = np.asarray(x, dtype=np.float32)
    kern = np.asarray(kernel, dtype=np.float32)

    in_maps = prepare_in_maps(x, kern)
    if _program_cache is None:
        _program_cache = _build_program()
    nc = _program_cache

    res = run_bass_kernel_spmd(nc, in_maps, core_ids=list(range(N_CORES)))
    outs = []
    for r in res.results:
        o = r["out"].reshape(NG, NS, 4, 32, W)[:, :, :, :M_STRIP, :]
        o = o.transpose(0, 2, 1, 3, 4)          # [g, i, s, m, c]
        body = o[:, :, :18].reshape(NIMG, 18 * M_STRIP, W)
        tail = o[:, :, 18, 504 - R0_LAST:, :].reshape(NIMG, H - 18 * M_STRIP,
                                                      W)
        outs.append(np.concatenate([body, tail], axis=1))
    out = np.concatenate(outs, axis=0)
    return out.reshape(B, C, H, W).astype(np.float32)
